# revision 1
# baseline (speedup 1.0000x reference)
"""Trainium2 Bass kernel for nn_Attention (B=2, T=2048, C=2048, H=16, causal, past_len=0).

Strategy: tensor-parallel over heads across 8 NeuronCores (2 heads/core).
  Phase 1 (qkv): each core computes q,k (transposed layout [hd, tok]) and v
    ([tok, hd]) for its 2 heads from the full token stream, in fp32r.
  Phase 2 (attention): per (batch, head): scoresT[k,q] = k.q/sqrt(hd) via PE,
    exp on ACT (no max-subtraction needed: scores are O(1)), causal mask by
    0/1 multiply on diagonal blocks, row-sums via a ones-matmul, out^T = v^T @
    attnT accumulated on PE, normalization by broadcasting 1/s across
    partitions.
  AllToAll: converts head-sharding -> token-sharding (each core ends up with
    all 16 heads' out^T for its 512 tokens). Split in two (one per local head)
    so the first A2A overlaps the second head's attention.
  Phase 3 (proj): y_slice[512, 2048] = out_slice @ proj_w.T computed locally;
    host concatenates the 8 slices.

All matmul operands are fp32r (fp32 with low 12 mantissa bits rounded away)
= full PE rate with ~1e-4 relative error. Host pre-rounds DRAM-sourced
operands; on-device producers (ACT/DVE copies) write fp32r directly.
"""
import sys
import numpy as np

if '/opt/trn_rl_repo' not in sys.path:
    sys.path.insert(0, '/opt/trn_rl_repo')

B, T, C, H, HD = 2, 2048, 2048, 16, 128
NCORES = 8
TOK = B * T            # 4096 global tokens
TSL = TOK // NCORES    # 512 tokens per core in the final output
SCALE = float(1.0 / np.sqrt(HD))

_CACHE = {}


def round_fp32r(x: np.ndarray) -> np.ndarray:
    """Round fp32 -> fp32r (drop low 12 mantissa bits, round-to-nearest-even)."""
    u = np.ascontiguousarray(x, dtype=np.float32).view(np.uint32)
    lsb = (u >> np.uint32(12)) & np.uint32(1)
    r = (u + np.uint32(0x7FF) + lsb) & np.uint32(0xFFFF_F000)
    return r.view(np.float32)


def build(debug_outputs=False):
    """Build the SPMD Bass program (same program on all 8 cores)."""
    import concourse.bacc as bacc
    import concourse.mybir as mybir
    from concourse import tile
    from contextlib import ExitStack

    f32 = mybir.dt.float32
    f32r = mybir.dt.float32r
    Exp = mybir.ActivationFunctionType.Exp

    nc = bacc.Bacc("TRN2", target_bir_lowering=False, debug=False,
                   num_devices=NCORES)

    xT_d = nc.dram_tensor("xT", [C, TOK], f32r, kind="ExternalInput")
    wq_d = nc.dram_tensor("wqkvT", [C, 768], f32r, kind="ExternalInput")
    pwT_d = nc.dram_tensor("pwT", [C, C], f32r, kind="ExternalInput")
    masks_d = nc.dram_tensor("masks", [4, 128, 512], f32r, kind="ExternalInput")
    ones_d = nc.dram_tensor("ones2", [128, 128], f32r, kind="ExternalInput")
    y_d = nc.dram_tensor("y", [TSL, C], f32, kind="ExternalOutput")
    if debug_outputs:
        dbg_qT = [nc.dram_tensor(f"dbg_qT{h}", [128, TOK], f32, kind="ExternalOutput") for h in range(2)]
        dbg_kT = [nc.dram_tensor(f"dbg_kT{h}", [128, TOK], f32, kind="ExternalOutput") for h in range(2)]
        dbg_v = nc.dram_tensor("dbg_v", [128, 32 * 256], f32, kind="ExternalOutput")
        dbg_ai = [nc.dram_tensor(f"dbg_ai{h}", [8 * 128, 512], f32, kind="ExternalOutput") for h in range(2)]

    with tile.TileContext(nc) as tc, ExitStack() as top:
        # ---- persistent pools
        sb_cst = top.enter_context(tc.tile_pool(name="cst", bufs=1))
        dram = top.enter_context(tc.tile_pool(name="dram", bufs=1, space="DRAM"))
        qkv_scope = top.enter_context(ExitStack())  # closed before phase 3
        sb_qkv = qkv_scope.enter_context(tc.tile_pool(name="qkv", bufs=1))

        qT = [sb_qkv.tile([128, TOK], f32r, name=f"qT{h}", tag=f"qT{h}") for h in range(2)]
        kT = [sb_qkv.tile([128, TOK], f32r, name=f"kT{h}", tag=f"kT{h}") for h in range(2)]
        v_sb = sb_qkv.tile([128, 32 * 256], f32r, name="v", tag="v")  # chunk ck at [:, ck*256:+256]

        mask_t = sb_cst.tile([128, 4 * 512], f32r, name="masks", tag="masks")
        ones_t = sb_cst.tile([128, 128], f32r, name="ones", tag="ones")

        a2a_in = [dram.tile([8 * 128, 512], f32, name=f"ai{i}", tag=f"ai{i}") for i in range(2)]
        a2a_out = [dram.tile([8 * 128, 512], f32, name=f"ao{i}", tag=f"ao{i}") for i in range(2)]

        # ================= Phase 1: qkv projection =================
        with ExitStack() as ph1, nc.named_scope("ph1_qkv"):
            sb_w = ph1.enter_context(tc.tile_pool(name="wq", bufs=1))
            sb_x = ph1.enter_context(tc.tile_pool(name="xs", bufs=3))
            ps_qk = ph1.enter_context(tc.tile_pool(name="pqk", bufs=1, space="PSUM"))
            ps_v = ph1.enter_context(tc.tile_pool(name="pv", bufs=1, space="PSUM"))

            wq_t = sb_w.tile([128, 16 * 768], f32r, name="wq", tag="wq")  # chunk c at [:, c*768:+768]

            for tb in range(8):  # 512-token blocks
                qk_ps = [ps_qk.tile([128, 512], f32, name=f"qk{f}", tag=f"qk{f}") for f in range(4)]
                v_ps = [ps_v.tile([128, 256], f32, name=f"v{s}", tag=f"v{s}") for s in range(4)]
                for cq in range(4):  # x loaded 4 c-chunks (1MB) per DMA
                    xt4 = sb_x.tile([128, 2048], f32r, name="xt4", tag="xt4")
                    if tb == 0:
                        # tb0: small per-chunk loads (low latency, spread
                        # across DMA queues) interleaved with weight chunks
                        for cc in range(4):
                            c = 4 * cq + cc
                            nc.sync.dma_start(wq_t[:, c * 768:(c + 1) * 768],
                                              wq_d[c * 128:(c + 1) * 128, :])
                            nc.sync.dma_start(xt4[:, cc * 512:(cc + 1) * 512],
                                              xT_d[c * 128:(c + 1) * 128, 0:512])
                    else:
                        nc.sync.dma_start(
                            xt4[:].rearrange("p (cc w) -> p cc w", cc=4),
                            xT_d[:, tb * 512:(tb + 1) * 512]
                               .rearrange("(c p) w -> p c w", p=128)[:, 4 * cq:4 * cq + 4, :])
                    for cc in range(4):
                        c = 4 * cq + cc
                        xt = xt4[:, cc * 512:(cc + 1) * 512]
                        w_c = wq_t[:, c * 768:(c + 1) * 768]
                        for f in range(4):  # q_h0, q_h1, k_h0, k_h1
                            nc.tensor.matmul(qk_ps[f][:], w_c[:, f * 128:(f + 1) * 128],
                                             xt, start=(c == 0), stop=(c == 15))
                        for s in range(4):  # v for 128-token sub-chunks
                            nc.tensor.matmul(v_ps[s][:],
                                             xt[:, s * 128:(s + 1) * 128],
                                             w_c[:, 512:768],
                                             start=(c == 0), stop=(c == 15))
                sl = slice(tb * 512, (tb + 1) * 512)
                nc.scalar.copy(qT[0][:, sl], qk_ps[0][:])
                nc.vector.tensor_copy(kT[0][:, sl], qk_ps[2][:])
                nc.scalar.copy(qT[1][:, sl], qk_ps[1][:])
                nc.vector.tensor_copy(kT[1][:, sl], qk_ps[3][:])
                for s in range(4):
                    ck = tb * 4 + s
                    nc.vector.tensor_copy(v_sb[:, ck * 256:(ck + 1) * 256],
                                          v_ps[s][:])

        # ================= Phase 2: attention =================
        with ExitStack() as ph2, nc.named_scope("ph2_attn"):
            ps_sc = ph2.enter_context(tc.tile_pool(name="psc", bufs=3, space="PSUM"))
            ps_o = ph2.enter_context(tc.tile_pool(name="po", bufs=3, space="PSUM"))
            ps_s = ph2.enter_context(tc.tile_pool(name="pss", bufs=2, space="PSUM"))
            sb_et = ph2.enter_context(tc.tile_pool(name="et", bufs=18))
            sb_sm = ph2.enter_context(tc.tile_pool(name="sm", bufs=3))
            sb_on = ph2.enter_context(tc.tile_pool(name="on", bufs=4))

            nc.sync.dma_start(mask_t[:].rearrange("p (o w) -> p o w", o=4),
                              masks_d[:].rearrange("o p w -> p o w"))
            nc.sync.dma_start(ones_t[:], ones_d[:])

            for idx, (b, hl) in enumerate([(0, 0), (1, 0), (0, 1), (1, 1)]):
                qTb = qT[hl][:, b * T:(b + 1) * T]
                kTb = kT[hl][:, b * T:(b + 1) * T]
                for g in range(4):  # query groups of 512
                    nk = 4 * (g + 1)
                    o_ps = ps_o.tile([128, 512], f32, name="o", tag="o")
                    # ones lhsT is [128,128]: every output partition gets the
                    # k-sum, i.e. the softmax denominator pre-broadcast.
                    s_ps = ps_s.tile([128, 512], f32, name="s", tag="s")
                    ets = []
                    for kj in range(nk):
                        sc_ps = ps_sc.tile([128, 512], f32, name="sc", tag="sc")
                        et = sb_et.tile([128, 512], f32r, name="et", tag="et")
                        nc.tensor.matmul(sc_ps[:], kTb[:, kj * 128:(kj + 1) * 128],
                                         qTb[:, g * 512:(g + 1) * 512],
                                         start=True, stop=True)
                        nc.scalar.activation(et[:], sc_ps[:], Exp, scale=SCALE)
                        if kj >= 4 * g:  # diagonal block: causal 0/1 mask
                            o = kj - 4 * g
                            nc.vector.tensor_mul(et[:], et[:],
                                                 mask_t[:, o * 512:(o + 1) * 512])
                        ck = b * 16 + kj
                        nc.tensor.matmul(o_ps[:],
                                         v_sb[:, ck * 256 + hl * 128:ck * 256 + (hl + 1) * 128],
                                         et[:], start=(kj == 0), stop=(kj == nk - 1))
                        ets.append(et)
                    # s-matmuls batched: consecutive mms share the ones
                    # stationary (no v/kT weight reloads interleaved)
                    for kj, et in enumerate(ets):
                        nc.tensor.matmul(s_ps[:], ones_t[:], et[:],
                                         start=(kj == 0), stop=(kj == nk - 1))
                    rs_bc = sb_sm.tile([128, 512], f32, name="rs_bc", tag="rs_bc")
                    nc.vector.reciprocal(rs_bc[:], s_ps[:])
                    on = sb_on.tile([128, 512], f32r, name="on", tag="on")
                    nc.vector.tensor_mul(on[:], o_ps[:], rs_bc[:])
                    dest = b * 4 + g
                    nc.sync.dma_start(a2a_in[hl][dest * 128:(dest + 1) * 128, :],
                                      on[:].bitcast(f32))
                if idx in (1, 3):  # both batches of this local head done
                    nc.gpsimd.collective_compute(
                        "AllToAll", mybir.AluOpType.bypass,
                        replica_groups=[list(range(NCORES))],
                        ins=[a2a_in[hl].opt()], outs=[a2a_out[hl].opt()],
                    )

        if debug_outputs:
            for h in range(2):
                nc.sync.dma_start(dbg_qT[h][:], qT[h][:].bitcast(f32))
                nc.sync.dma_start(dbg_kT[h][:], kT[h][:].bitcast(f32))
                nc.sync.dma_start(dbg_ai[h][:], a2a_in[h][:])
            nc.sync.dma_start(dbg_v[:], v_sb[:].bitcast(f32))

        # ================= Phase 3: output projection =================
        qkv_scope.close()  # release qT/kT/v SBUF for phase 3
        with ExitStack() as ph3, nc.named_scope("ph3_proj"):
            sb_ot = ph3.enter_context(tc.tile_pool(name="ot", bufs=1))
            sb_pw = ph3.enter_context(tc.tile_pool(name="pw", bufs=1))
            sb_y = ph3.enter_context(tc.tile_pool(name="ysb", bufs=3))
            ps_y = ph3.enter_context(tc.tile_pool(name="py", bufs=2, space="PSUM"))

            # Two passes so the PE's in-order queue never blocks on A2A#2
            # while even-head (A2A#1) work remains: pass A accumulates heads
            # from A2A#1 into an SBUF partial, pass B adds the A2A#2 heads.
            # ot loads for pass B are issued after pass A's loads so the
            # in-order SP queue doesn't block on A2A#2 either.
            y_acc = sb_ot.tile([128, 4 * 4 * 512], f32, name="yacc", tag="yacc")
            for pa in range(2):
                tiles = []  # (ot, pwt_full) per head of this pass
                for m in range(8 * pa, 8 * pa + 8):
                    hl, blk = (0, m) if m < 8 else (1, m - 8)
                    gh = 2 * blk + hl
                    ot = sb_ot.tile([128, 512], f32r, name=f"ot{m}", tag=f"ot{m}")
                    nc.sync.dma_start(ot[:], a2a_out[hl][blk * 128:(blk + 1) * 128, :].bitcast(f32r))
                    pwt = sb_pw.tile([128, 2048], f32r, name=f"pwt{m % 8}", tag=f"pwt{m % 8}")
                    nc.sync.dma_start(pwt[:], pwT_d[gh * 128:(gh + 1) * 128, :])
                    tiles.append((ot, pwt))
                for db in range(4):  # 512-wide output column blocks
                    y_ps = [ps_y.tile([128, 512], f32, name=f"y{t_}", tag=f"y{t_}")
                            for t_ in range(4)]
                    for mi, (ot, pwt) in enumerate(tiles):
                        for t_ in range(4):
                            nc.tensor.matmul(y_ps[t_][:], ot[:, t_ * 128:(t_ + 1) * 128],
                                             pwt[:, db * 512:(db + 1) * 512],
                                             start=(mi == 0), stop=(mi == 7))
                    for t_ in range(4):
                        acc = y_acc[:, (db * 4 + t_) * 512:(db * 4 + t_ + 1) * 512]
                        if pa == 0:
                            nc.scalar.copy(acc, y_ps[t_][:])
                        else:
                            y_sb = sb_y.tile([128, 512], f32, name="ysb", tag="ysb")
                            nc.vector.tensor_add(y_sb[:], y_ps[t_][:], acc)
                            nc.sync.dma_start(y_d[t_ * 128:(t_ + 1) * 128,
                                                  db * 512:(db + 1) * 512], y_sb[:])

    nc.finalize()
    return nc


def prep_in_maps(x, qkv_w, proj_w):
    """Host-side sharding + fp32r pre-rounding. Returns per-core input maps."""
    x = np.ascontiguousarray(np.asarray(x, dtype=np.float32).reshape(TOK, C))
    qkv_w = np.asarray(qkv_w, dtype=np.float32)
    proj_w = np.asarray(proj_w, dtype=np.float32)

    xT = round_fp32r(x.T)                       # [C, TOK], shared
    pwT = round_fp32r(proj_w.T)                 # [C, C], shared
    masks = np.zeros((4, 128, 512), dtype=np.float32)
    kj_i = np.arange(128)[:, None]
    qi_i = np.arange(512)[None, :]
    for o in range(4):
        masks[o] = (kj_i + o * 128 <= qi_i).astype(np.float32)
    ones2 = np.ones((128, 128), dtype=np.float32)

    in_maps = []
    for i in range(NCORES):
        r0 = 2 * i * HD
        rows = np.concatenate([
            qkv_w[r0:r0 + 2 * HD],              # q rows, heads 2i, 2i+1
            qkv_w[C + r0:C + r0 + 2 * HD],      # k rows
            qkv_w[2 * C + r0:2 * C + r0 + 2 * HD],  # v rows
        ], axis=0)                              # [768, C]
        wqkvT = round_fp32r(rows.T)             # [C, 768]
        in_maps.append({"xT": xT, "wqkvT": wqkvT, "pwT": pwT,
                        "masks": masks, "ones2": ones2})
    return in_maps


def kernel(x, qkv_w, proj_w, past=None, past_len=0, **_ignored):
    # past is fully overwritten before being read (past_len == 0), so the
    # output does not depend on it.
    from concourse.bass_utils import run_bass_kernel_spmd
    nc = _CACHE.get("nc")
    if nc is None:
        nc = _CACHE["nc"] = build()
    in_maps = prep_in_maps(x, qkv_w, proj_w)
    res = run_bass_kernel_spmd(nc, in_maps, list(range(NCORES)))
    y = np.concatenate([res.results[i]["y"] for i in range(NCORES)], axis=0)
    return np.ascontiguousarray(y.reshape(B, T, C), dtype=np.float32)



# revision 6
# speedup vs baseline: 1.1883x; 1.1883x over previous
"""Trainium2 Bass kernel for nn_Attention (B=2, T=2048, C=2048, H=16, causal, past_len=0).

Strategy: tensor-parallel over heads across 8 NeuronCores (2 heads/core).
  Phase 1 (qkv): each core computes q,k (transposed layout [hd, tok]) and v
    ([tok, hd]) for its 2 heads from the full token stream.
  Phase 2 (attention): per (batch, head): scoresT[k,q] = k.q/sqrt(hd) via PE,
    exp on ACT (no max-subtraction needed: scores are O(1)), causal mask by
    0/1 multiply on diagonal blocks, row-sums via a ones-matmul, out^T = v^T @
    attnT accumulated on PE, normalization by broadcasting 1/s across
    partitions (reciprocal_approx_fast on DVE, ~18 good bits).
  AllToAll: converts head-sharding -> token-sharding (each core ends up with
    all 16 heads' out^T for its 512 tokens). Split in two (one per local head)
    so the first A2A overlaps the second head's attention; bf16 payload.
  Phase 3 (proj): y_slice[512, 2048] = out_slice @ proj_w.T computed locally
    from SBUF-prefetched proj weights; host concatenates the 8 slices.

All matmul operands are bf16 (fp32 PSUM accumulation): full PE rate, FWL
weight loads (hidden behind matmuls), half the SBUF/HBM/collective traffic
of fp32. End-to-end relative error ~6e-3 (gate 2e-2).
"""
import sys
import numpy as np

if '/opt/trn_rl_repo' not in sys.path:
    sys.path.insert(0, '/opt/trn_rl_repo')

B, T, C, H, HD = 2, 2048, 2048, 16, 128
NCORES = 8
TOK = B * T            # 4096 global tokens
TSL = TOK // NCORES    # 512 tokens per core in the final output
SCALE = float(1.0 / np.sqrt(HD))

_CACHE = {}


def build():
    """Build the SPMD Bass program (same program on all 8 cores)."""
    import concourse.bacc as bacc
    import concourse.mybir as mybir
    from concourse import tile
    from contextlib import ExitStack

    f32 = mybir.dt.float32
    bf16 = mybir.dt.bfloat16
    Exp = mybir.ActivationFunctionType.Exp

    nc = bacc.Bacc("TRN2", target_bir_lowering=False, debug=False,
                   num_devices=NCORES)

    xT_d = nc.dram_tensor("xT", [C, TOK], bf16, kind="ExternalInput")
    wq_d = nc.dram_tensor("wqkvT", [C, 768], bf16, kind="ExternalInput")
    pwT_d = nc.dram_tensor("pwT", [C, C], bf16, kind="ExternalInput")
    masks_d = nc.dram_tensor("masks", [4, 128, 512], bf16, kind="ExternalInput")
    ones_d = nc.dram_tensor("ones2", [128, 128], bf16, kind="ExternalInput")
    y_d = nc.dram_tensor("y", [TSL, C], f32, kind="ExternalOutput")

    with tile.TileContext(nc) as tc, ExitStack() as top:
        # ---- persistent pools
        sb_cst = top.enter_context(tc.tile_pool(name="cst", bufs=1))
        dram = top.enter_context(tc.tile_pool(name="dram", bufs=1, space="DRAM"))
        qkv_scope = top.enter_context(ExitStack())  # closed before phase 3
        sb_qkv = qkv_scope.enter_context(tc.tile_pool(name="qkv", bufs=1))

        qT = [sb_qkv.tile([128, TOK], bf16, name=f"qT{h}", tag=f"qT{h}") for h in range(2)]
        kT = [sb_qkv.tile([128, TOK], bf16, name=f"kT{h}", tag=f"kT{h}") for h in range(2)]
        v_sb = sb_qkv.tile([128, 32 * 256], bf16, name="v", tag="v")  # chunk ck at [:, ck*256:+256]

        mask_t = sb_cst.tile([128, 4 * 512], bf16, name="masks", tag="masks")
        ones_t = sb_cst.tile([128, 128], bf16, name="ones", tag="ones")
        # proj weights live in SBUF from mid-kernel on (prefetched in phase 2)
        pw_sb = sb_cst.tile([128, 16 * 2048], bf16, name="pw", tag="pw")

        a2a_in = [dram.tile([8 * 128, 512], bf16, name=f"ai{i}", tag=f"ai{i}") for i in range(2)]
        a2a_out = [dram.tile([8 * 128, 512], bf16, name=f"ao{i}", tag=f"ao{i}") for i in range(2)]

        # ================= Phase 1: qkv projection =================
        with ExitStack() as ph1, nc.named_scope("ph1_qkv"):
            sb_w = ph1.enter_context(tc.tile_pool(name="wq", bufs=1))
            sb_x = ph1.enter_context(tc.tile_pool(name="xs", bufs=3))
            ps_qk = ph1.enter_context(tc.tile_pool(name="pqk", bufs=1, space="PSUM"))
            ps_v = ph1.enter_context(tc.tile_pool(name="pv", bufs=1, space="PSUM"))

            wq_t = sb_w.tile([128, 16 * 768], bf16, name="wq", tag="wq")  # chunk c at [:, c*768:+768]

            for tb in range(8):  # 512-token blocks
                qk_ps = [ps_qk.tile([128, 512], f32, name=f"qk{f}", tag=f"qk{f}") for f in range(4)]
                v_ps = [ps_v.tile([128, 256], f32, name=f"v{s}", tag=f"v{s}") for s in range(4)]
                for cq in range(4):  # x loaded 4 c-chunks per DMA
                    xt4 = sb_x.tile([128, 2048], bf16, name="xt4", tag="xt4")
                    if tb == 0:
                        # tb0: small per-chunk loads (low latency, spread
                        # across DMA queues) interleaved with weight chunks
                        for cc in range(4):
                            c = 4 * cq + cc
                            nc.sync.dma_start(wq_t[:, c * 768:(c + 1) * 768],
                                              wq_d[c * 128:(c + 1) * 128, :])
                            nc.sync.dma_start(xt4[:, cc * 512:(cc + 1) * 512],
                                              xT_d[c * 128:(c + 1) * 128, 0:512])
                    else:
                        nc.sync.dma_start(
                            xt4[:].rearrange("p (cc w) -> p cc w", cc=4),
                            xT_d[:, tb * 512:(tb + 1) * 512]
                               .rearrange("(c p) w -> p c w", p=128)[:, 4 * cq:4 * cq + 4, :])
                    for cc in range(4):
                        c = 4 * cq + cc
                        xt = xt4[:, cc * 512:(cc + 1) * 512]
                        w_c = wq_t[:, c * 768:(c + 1) * 768]
                        for f in range(4):  # q_h0, q_h1, k_h0, k_h1
                            nc.tensor.matmul(qk_ps[f][:], w_c[:, f * 128:(f + 1) * 128],
                                             xt, start=(c == 0), stop=(c == 15))
                        for s in range(4):  # v for 128-token sub-chunks
                            nc.tensor.matmul(v_ps[s][:],
                                             xt[:, s * 128:(s + 1) * 128],
                                             w_c[:, 512:768],
                                             start=(c == 0), stop=(c == 15))
                sl = slice(tb * 512, (tb + 1) * 512)
                nc.scalar.copy(qT[0][:, sl], qk_ps[0][:])
                nc.vector.tensor_copy(kT[0][:, sl], qk_ps[2][:])
                nc.scalar.copy(qT[1][:, sl], qk_ps[1][:])
                nc.vector.tensor_copy(kT[1][:, sl], qk_ps[3][:])
                for s in range(4):
                    ck = tb * 4 + s
                    # ACT engine: keeps the DVE queue short so the psum tiles
                    # free up before the next tb's first v matmul
                    nc.scalar.copy(v_sb[:, ck * 256:(ck + 1) * 256],
                                   v_ps[s][:])

        # ================= Phase 2: attention =================
        with ExitStack() as ph2, nc.named_scope("ph2_attn"):
            ps_sc = ph2.enter_context(tc.tile_pool(name="psc", bufs=3, space="PSUM"))
            ps_o = ph2.enter_context(tc.tile_pool(name="po", bufs=3, space="PSUM"))
            ps_s = ph2.enter_context(tc.tile_pool(name="pss", bufs=2, space="PSUM"))
            sb_et = ph2.enter_context(tc.tile_pool(name="et", bufs=18))
            sb_sm = ph2.enter_context(tc.tile_pool(name="sm", bufs=3))
            sb_on = ph2.enter_context(tc.tile_pool(name="on", bufs=4))

            nc.sync.dma_start(mask_t[:].rearrange("p (o w) -> p o w", o=4),
                              masks_d[:].rearrange("o p w -> p o w"))
            nc.sync.dma_start(ones_t[:], ones_d[:])
            # prefetch proj weights while DMA is otherwise idle (phase 2 has
            # no bulk loads); ready long before phase 3 needs them
            for gh in range(16):
                nc.sync.dma_start(pw_sb[:, gh * 2048:(gh + 1) * 2048],
                                  pwT_d[gh * 128:(gh + 1) * 128, :])

            for idx, (b, hl) in enumerate([(0, 0), (1, 0), (0, 1), (1, 1)]):
                qTb = qT[hl][:, b * T:(b + 1) * T]
                kTb = kT[hl][:, b * T:(b + 1) * T]
                for g in range(4):  # query groups of 512
                    nk = 4 * (g + 1)
                    o_ps = ps_o.tile([128, 512], f32, name="o", tag="o")
                    # ones lhsT is [128,128]: every output partition gets the
                    # k-sum, i.e. the softmax denominator pre-broadcast.
                    s_ps = ps_s.tile([128, 512], f32, name="s", tag="s")
                    ets = []
                    for kj in range(nk):
                        sc_ps = ps_sc.tile([128, 512], f32, name="sc", tag="sc")
                        et = sb_et.tile([128, 512], bf16, name="et", tag="et")
                        nc.tensor.matmul(sc_ps[:], kTb[:, kj * 128:(kj + 1) * 128],
                                         qTb[:, g * 512:(g + 1) * 512],
                                         start=True, stop=True)
                        nc.scalar.activation(et[:], sc_ps[:], Exp, scale=SCALE)
                        if kj >= 4 * g:  # diagonal block: causal 0/1 mask
                            o = kj - 4 * g
                            nc.vector.tensor_mul(et[:], et[:],
                                                 mask_t[:, o * 512:(o + 1) * 512])
                        ck = b * 16 + kj
                        nc.tensor.matmul(o_ps[:],
                                         v_sb[:, ck * 256 + hl * 128:ck * 256 + (hl + 1) * 128],
                                         et[:], start=(kj == 0), stop=(kj == nk - 1))
                        ets.append(et)
                    # s-matmuls batched: consecutive mms share the ones
                    # stationary (no v/kT weight reloads interleaved)
                    for kj, et in enumerate(ets):
                        nc.tensor.matmul(s_ps[:], ones_t[:], et[:],
                                         start=(kj == 0), stop=(kj == nk - 1))
                    rs_bc = sb_sm.tile([128, 512], f32, name="rs_bc", tag="rs_bc")
                    nc.vector.reciprocal_approx_fast(rs_bc[:], s_ps[:])
                    on = sb_on.tile([128, 512], bf16, name="on", tag="on")
                    nc.vector.tensor_mul(on[:], o_ps[:], rs_bc[:])
                    dest = b * 4 + g
                    nc.sync.dma_start(a2a_in[hl][dest * 128:(dest + 1) * 128, :],
                                      on[:])
                if idx in (1, 3):  # both batches of this local head done
                    nc.gpsimd.collective_compute(
                        "AllToAll", mybir.AluOpType.bypass,
                        replica_groups=[list(range(NCORES))],
                        ins=[a2a_in[hl].opt()], outs=[a2a_out[hl].opt()],
                    )

        # ================= Phase 3: output projection =================
        qkv_scope.close()  # release qT/kT/v SBUF for phase 3
        with ExitStack() as ph3, nc.named_scope("ph3_proj"):
            sb_ot = ph3.enter_context(tc.tile_pool(name="ot", bufs=1))
            sb_y = ph3.enter_context(tc.tile_pool(name="ysb", bufs=3))
            ps_y = ph3.enter_context(tc.tile_pool(name="py", bufs=2, space="PSUM"))

            # Two passes so the PE's in-order queue never blocks on A2A#2
            # while even-head (A2A#1) work remains: pass A accumulates heads
            # from A2A#1 into an SBUF partial, pass B adds the A2A#2 heads.
            # ot loads for pass B are issued after pass A's loads so the
            # in-order SP queue doesn't block on A2A#2 either.
            y_acc = sb_ot.tile([128, 4 * 4 * 512], f32, name="yacc", tag="yacc")
            for pa in range(2):
                tiles = []  # (ot, gh) per head of this pass
                for m in range(8 * pa, 8 * pa + 8):
                    hl, blk = (0, m) if m < 8 else (1, m - 8)
                    gh = 2 * blk + hl
                    ot = sb_ot.tile([128, 512], bf16, name=f"ot{m}", tag=f"ot{m}")
                    nc.sync.dma_start(ot[:], a2a_out[hl][blk * 128:(blk + 1) * 128, :])
                    tiles.append((ot, gh))
                for db in range(4):  # 512-wide output column blocks
                    y_ps = [ps_y.tile([128, 512], f32, name=f"y{t_}", tag=f"y{t_}")
                            for t_ in range(4)]
                    for mi, (ot, gh) in enumerate(tiles):
                        pwt = pw_sb[:, gh * 2048:(gh + 1) * 2048]
                        for t_ in range(4):
                            nc.tensor.matmul(y_ps[t_][:], ot[:, t_ * 128:(t_ + 1) * 128],
                                             pwt[:, db * 512:(db + 1) * 512],
                                             start=(mi == 0), stop=(mi == 7))
                    for t_ in range(4):
                        acc = y_acc[:, (db * 4 + t_) * 512:(db * 4 + t_ + 1) * 512]
                        if pa == 0:
                            nc.scalar.copy(acc, y_ps[t_][:])
                        else:
                            y_sb = sb_y.tile([128, 512], f32, name="ysb", tag="ysb")
                            nc.vector.tensor_add(y_sb[:], y_ps[t_][:], acc)
                            nc.sync.dma_start(y_d[t_ * 128:(t_ + 1) * 128,
                                                  db * 512:(db + 1) * 512], y_sb[:])

    nc.finalize()
    return nc


def prep_in_maps(x, qkv_w, proj_w):
    """Host-side sharding + bf16 pre-rounding. Returns per-core input maps."""
    import ml_dtypes
    bf = ml_dtypes.bfloat16

    x = np.ascontiguousarray(np.asarray(x, dtype=np.float32).reshape(TOK, C))
    qkv_w = np.asarray(qkv_w, dtype=np.float32)
    proj_w = np.asarray(proj_w, dtype=np.float32)

    xT = np.ascontiguousarray(x.T).astype(bf)     # [C, TOK], shared
    pwT = np.ascontiguousarray(proj_w.T).astype(bf)  # [C, C], shared
    masks = np.zeros((4, 128, 512), dtype=np.float32)
    kj_i = np.arange(128)[:, None]
    qi_i = np.arange(512)[None, :]
    for o in range(4):
        masks[o] = (kj_i + o * 128 <= qi_i).astype(np.float32)
    masks = masks.astype(bf)
    ones2 = np.ones((128, 128), dtype=bf)

    in_maps = []
    for i in range(NCORES):
        r0 = 2 * i * HD
        rows = np.concatenate([
            qkv_w[r0:r0 + 2 * HD],              # q rows, heads 2i, 2i+1
            qkv_w[C + r0:C + r0 + 2 * HD],      # k rows
            qkv_w[2 * C + r0:2 * C + r0 + 2 * HD],  # v rows
        ], axis=0)                              # [768, C]
        wqkvT = np.ascontiguousarray(rows.T).astype(bf)  # [C, 768]
        in_maps.append({"xT": xT, "wqkvT": wqkvT, "pwT": pwT,
                        "masks": masks, "ones2": ones2})
    return in_maps


def kernel(x, qkv_w, proj_w, past=None, past_len=0, **_ignored):
    # past is fully overwritten before being read (past_len == 0), so the
    # output does not depend on it.
    from concourse.bass_utils import run_bass_kernel_spmd
    nc = _CACHE.get("nc")
    if nc is None:
        nc = _CACHE["nc"] = build()
    in_maps = prep_in_maps(x, qkv_w, proj_w)
    res = run_bass_kernel_spmd(nc, in_maps, list(range(NCORES)))
    y = np.concatenate([res.results[i]["y"] for i in range(NCORES)], axis=0)
    return np.ascontiguousarray(y.reshape(B, T, C), dtype=np.float32)


# revision 7
# speedup vs baseline: 1.2382x; 1.0420x over previous
"""Trainium2 Bass kernel for nn_Attention (B=2, T=2048, C=2048, H=16, causal, past_len=0).

Strategy: tensor-parallel over heads across 8 NeuronCores (2 heads/core).
  Phase 1 (qkv): each core computes q,k (transposed layout [hd, tok]) and v
    ([tok, hd]) for its 2 heads from the full token stream.
  Phase 2 (attention): per (batch, head): scoresT[k,q] = k.q/sqrt(hd) via PE,
    exp on ACT (no max-subtraction needed: scores are O(1)), causal mask by
    0/1 multiply on diagonal blocks, row-sums via a ones-matmul, out^T = v^T @
    attnT accumulated on PE, normalization by broadcasting 1/s across
    partitions (reciprocal_approx_fast on DVE, ~18 good bits).
  AllToAll: converts head-sharding -> token-sharding (each core ends up with
    all 16 heads' out^T for its 512 tokens). Split in two (one per local head)
    so the first A2A overlaps the second head's attention; bf16 payload.
  Phase 3 (proj): y_slice[512, 2048] = out_slice @ proj_w.T computed locally
    from SBUF-prefetched proj weights; host concatenates the 8 slices.

All matmul operands are bf16 (fp32 PSUM accumulation): full PE rate, FWL
weight loads (hidden behind matmuls), half the SBUF/HBM/collective traffic
of fp32. End-to-end relative error ~6e-3 (gate 2e-2).
"""
import sys
import numpy as np

if '/opt/trn_rl_repo' not in sys.path:
    sys.path.insert(0, '/opt/trn_rl_repo')

B, T, C, H, HD = 2, 2048, 2048, 16, 128
NCORES = 8
TOK = B * T            # 4096 global tokens
TSL = TOK // NCORES    # 512 tokens per core in the final output
SCALE = float(1.0 / np.sqrt(HD))

_CACHE = {}


def build():
    """Build the SPMD Bass program (same program on all 8 cores)."""
    import concourse.bacc as bacc
    import concourse.mybir as mybir
    from concourse import tile
    from contextlib import ExitStack

    f32 = mybir.dt.float32
    bf16 = mybir.dt.bfloat16
    Exp = mybir.ActivationFunctionType.Exp

    nc = bacc.Bacc("TRN2", target_bir_lowering=False, debug=False,
                   num_devices=NCORES)

    xT_d = nc.dram_tensor("xT", [C, TOK], bf16, kind="ExternalInput")
    wq_d = nc.dram_tensor("wqkvT", [C, 768], bf16, kind="ExternalInput")
    pwT_d = nc.dram_tensor("pwT", [C, C], bf16, kind="ExternalInput")
    masks_d = nc.dram_tensor("masks", [4, 128, 512], bf16, kind="ExternalInput")
    ones_d = nc.dram_tensor("ones2", [128, 128], bf16, kind="ExternalInput")
    y_d = nc.dram_tensor("y", [TSL, C], f32, kind="ExternalOutput")

    with tile.TileContext(nc) as tc, ExitStack() as top:
        # ---- persistent pools
        sb_cst = top.enter_context(tc.tile_pool(name="cst", bufs=1))
        dram = top.enter_context(tc.tile_pool(name="dram", bufs=1, space="DRAM"))
        qkv_scope = top.enter_context(ExitStack())  # closed before phase 3
        sb_qkv = qkv_scope.enter_context(tc.tile_pool(name="qkv", bufs=1))

        qT = [sb_qkv.tile([128, TOK], bf16, name=f"qT{h}", tag=f"qT{h}") for h in range(2)]
        kT = [sb_qkv.tile([128, TOK], bf16, name=f"kT{h}", tag=f"kT{h}") for h in range(2)]
        v_sb = sb_qkv.tile([128, 32 * 256], bf16, name="v", tag="v")  # chunk ck at [:, ck*256:+256]

        mask_t = sb_cst.tile([128, 4 * 512], bf16, name="masks", tag="masks")
        ones_t = sb_cst.tile([128, 128], bf16, name="ones", tag="ones")
        # proj weights live in SBUF from mid-kernel on (prefetched in phase 2)
        pw_sb = sb_cst.tile([128, 16 * 2048], bf16, name="pw", tag="pw")

        a2a_in = [dram.tile([8 * 128, 512], bf16, name=f"ai{i}", tag=f"ai{i}") for i in range(2)]
        a2a_out = [dram.tile([8 * 128, 512], bf16, name=f"ao{i}", tag=f"ao{i}") for i in range(2)]

        # ================= Phase 1: qkv projection =================
        with ExitStack() as ph1, nc.named_scope("ph1_qkv"):
            sb_w = ph1.enter_context(tc.tile_pool(name="wq", bufs=1))
            sb_x = ph1.enter_context(tc.tile_pool(name="xs", bufs=3))
            ps_qk = ph1.enter_context(tc.tile_pool(name="pqk", bufs=1, space="PSUM"))
            ps_v = ph1.enter_context(tc.tile_pool(name="pv", bufs=1, space="PSUM"))

            wq_t = sb_w.tile([128, 16 * 768], bf16, name="wq", tag="wq")  # chunk c at [:, c*768:+768]

            for tb in range(8):  # 512-token blocks
                qk_ps = [ps_qk.tile([128, 512], f32, name=f"qk{f}", tag=f"qk{f}") for f in range(4)]
                v_ps = [ps_v.tile([128, 256], f32, name=f"v{s}", tag=f"v{s}") for s in range(4)]
                for cq in range(4):  # x loaded 4 c-chunks per DMA
                    xt4 = sb_x.tile([128, 2048], bf16, name="xt4", tag="xt4")
                    if tb == 0:
                        # tb0: small per-chunk loads (low latency, spread
                        # across DMA queues) interleaved with weight chunks
                        for cc in range(4):
                            c = 4 * cq + cc
                            nc.sync.dma_start(wq_t[:, c * 768:(c + 1) * 768],
                                              wq_d[c * 128:(c + 1) * 128, :])
                            nc.sync.dma_start(xt4[:, cc * 512:(cc + 1) * 512],
                                              xT_d[c * 128:(c + 1) * 128, 0:512])
                    else:
                        nc.sync.dma_start(
                            xt4[:].rearrange("p (cc w) -> p cc w", cc=4),
                            xT_d[:, tb * 512:(tb + 1) * 512]
                               .rearrange("(c p) w -> p c w", p=128)[:, 4 * cq:4 * cq + 4, :])
                    for cc in range(4):
                        c = 4 * cq + cc
                        xt = xt4[:, cc * 512:(cc + 1) * 512]
                        w_c = wq_t[:, c * 768:(c + 1) * 768]
                        for f in range(4):  # q_h0, q_h1, k_h0, k_h1
                            nc.tensor.matmul(qk_ps[f][:], w_c[:, f * 128:(f + 1) * 128],
                                             xt, start=(c == 0), stop=(c == 15))
                        for s in range(4):  # v for 128-token sub-chunks
                            nc.tensor.matmul(v_ps[s][:],
                                             xt[:, s * 128:(s + 1) * 128],
                                             w_c[:, 512:768],
                                             start=(c == 0), stop=(c == 15))
                sl = slice(tb * 512, (tb + 1) * 512)
                nc.scalar.copy(qT[0][:, sl], qk_ps[0][:])
                nc.vector.tensor_copy(kT[0][:, sl], qk_ps[2][:])
                nc.scalar.copy(qT[1][:, sl], qk_ps[1][:])
                nc.vector.tensor_copy(kT[1][:, sl], qk_ps[3][:])
                for s in range(4):
                    ck = tb * 4 + s
                    # ACT engine: keeps the DVE queue short so the psum tiles
                    # free up before the next tb's first v matmul
                    nc.scalar.copy(v_sb[:, ck * 256:(ck + 1) * 256],
                                   v_ps[s][:])

        # ================= Phase 2: attention =================
        with ExitStack() as ph2, nc.named_scope("ph2_attn"):
            ps_sc = ph2.enter_context(tc.tile_pool(name="psc", bufs=2, space="PSUM"))
            ps_o = ph2.enter_context(tc.tile_pool(name="po", bufs=3, space="PSUM"))
            ps_s = ph2.enter_context(tc.tile_pool(name="pss", bufs=1, space="PSUM"))
            sb_et = ph2.enter_context(tc.tile_pool(name="et", bufs=12))
            sb_ts = ph2.enter_context(tc.tile_pool(name="ts", bufs=24))
            sb_sm = ph2.enter_context(tc.tile_pool(name="sm", bufs=3))
            sb_on = ph2.enter_context(tc.tile_pool(name="on", bufs=4))

            nc.sync.dma_start(mask_t[:].rearrange("p (o w) -> p o w", o=4),
                              masks_d[:].rearrange("o p w -> p o w"))
            nc.sync.dma_start(ones_t[:], ones_d[:])
            # prefetch proj weights while DMA is otherwise idle (phase 2 has
            # no bulk loads); ready long before phase 3 needs them
            for gh in range(16):
                nc.sync.dma_start(pw_sb[:, gh * 2048:(gh + 1) * 2048],
                                  pwT_d[gh * 128:(gh + 1) * 128, :])

            for idx, (b, hl) in enumerate([(0, 0), (1, 0), (0, 1), (1, 1)]):
                qTb = qT[hl][:, b * T:(b + 1) * T]
                kTb = kT[hl][:, b * T:(b + 1) * T]
                for g in range(4):  # query groups of 512
                    nk = 4 * (g + 1)
                    qg = qTb[:, g * 512:(g + 1) * 512]
                    o_ps = ps_o.tile([128, 512], f32, name="o", tag="o")
                    ets = []  # [128,512] views of the masked exp'd blocks
                    for p in range(nk // 2):  # k-block pairs
                        # two k-blocks share one 2-bank psum tile (each MM's
                        # group is bank-aligned) so exp runs as one wide ACT op
                        sc_ps = ps_sc.tile([128, 1024], f32, name="sc", tag="sc")
                        et = sb_et.tile([128, 1024], bf16, name="et", tag="et")
                        for half in range(2):
                            kj = 2 * p + half
                            nc.tensor.matmul(sc_ps[:, half * 512:(half + 1) * 512],
                                             kTb[:, kj * 128:(kj + 1) * 128],
                                             qg, start=True, stop=True)
                        nc.scalar.activation(et[:], sc_ps[:], Exp, scale=SCALE)
                        for half in range(2):
                            kj = 2 * p + half
                            eth = et[:, half * 512:(half + 1) * 512]
                            if kj >= 4 * g:  # diagonal block: causal 0/1 mask
                                o = kj - 4 * g
                                nc.vector.tensor_mul(eth, eth,
                                                     mask_t[:, o * 512:(o + 1) * 512])
                            ck = b * 16 + kj
                            nc.tensor.matmul(o_ps[:],
                                             v_sb[:, ck * 256 + hl * 128:ck * 256 + (hl + 1) * 128],
                                             eth, start=(kj == 0), stop=(kj == nk - 1))
                            ets.append(eth)
                    # softmax denominator: bf16 tree-sum of the et blocks on
                    # DVE (error ~0.07% max on the sum), then a single
                    # ones-matmul for the partition reduction + broadcast
                    # (every output partition gets the k-sum). Saves nk-1
                    # 512-col PE matmuls per group vs per-block s-matmuls.
                    lvl = ets
                    while len(lvl) > 1:
                        nxt = []
                        for i in range(0, len(lvl) - 1, 2):
                            ts = sb_ts.tile([128, 512], bf16, name="ts", tag="ts")
                            nc.vector.tensor_add(ts[:], lvl[i], lvl[i + 1])
                            nxt.append(ts[:])
                        if len(lvl) % 2:
                            nxt.append(lvl[-1])
                        lvl = nxt
                    s_ps = ps_s.tile([128, 512], f32, name="s", tag="s")
                    nc.tensor.matmul(s_ps[:], ones_t[:], lvl[0],
                                     start=True, stop=True)
                    rs_bc = sb_sm.tile([128, 512], f32, name="rs_bc", tag="rs_bc")
                    nc.vector.reciprocal_approx_fast(rs_bc[:], s_ps[:])
                    on = sb_on.tile([128, 512], bf16, name="on", tag="on")
                    nc.vector.tensor_mul(on[:], o_ps[:], rs_bc[:])
                    dest = b * 4 + g
                    nc.sync.dma_start(a2a_in[hl][dest * 128:(dest + 1) * 128, :],
                                      on[:])
                if idx in (1, 3):  # both batches of this local head done
                    nc.gpsimd.collective_compute(
                        "AllToAll", mybir.AluOpType.bypass,
                        replica_groups=[list(range(NCORES))],
                        ins=[a2a_in[hl].opt()], outs=[a2a_out[hl].opt()],
                    )

        # ================= Phase 3: output projection =================
        qkv_scope.close()  # release qT/kT/v SBUF for phase 3
        with ExitStack() as ph3, nc.named_scope("ph3_proj"):
            sb_ot = ph3.enter_context(tc.tile_pool(name="ot", bufs=1))
            sb_y = ph3.enter_context(tc.tile_pool(name="ysb", bufs=3))
            ps_y = ph3.enter_context(tc.tile_pool(name="py", bufs=2, space="PSUM"))

            # Two passes so the PE's in-order queue never blocks on A2A#2
            # while even-head (A2A#1) work remains: pass A accumulates heads
            # from A2A#1 into an SBUF partial, pass B adds the A2A#2 heads.
            # ot loads for pass B are issued after pass A's loads so the
            # in-order SP queue doesn't block on A2A#2 either.
            y_acc = sb_ot.tile([128, 4 * 4 * 512], f32, name="yacc", tag="yacc")
            for pa in range(2):
                tiles = []  # (ot, gh) per head of this pass
                for m in range(8 * pa, 8 * pa + 8):
                    hl, blk = (0, m) if m < 8 else (1, m - 8)
                    gh = 2 * blk + hl
                    ot = sb_ot.tile([128, 512], bf16, name=f"ot{m}", tag=f"ot{m}")
                    nc.sync.dma_start(ot[:], a2a_out[hl][blk * 128:(blk + 1) * 128, :])
                    tiles.append((ot, gh))
                for db in range(4):  # 512-wide output column blocks
                    y_ps = [ps_y.tile([128, 512], f32, name=f"y{t_}", tag=f"y{t_}")
                            for t_ in range(4)]
                    for mi, (ot, gh) in enumerate(tiles):
                        pwt = pw_sb[:, gh * 2048:(gh + 1) * 2048]
                        for t_ in range(4):
                            nc.tensor.matmul(y_ps[t_][:], ot[:, t_ * 128:(t_ + 1) * 128],
                                             pwt[:, db * 512:(db + 1) * 512],
                                             start=(mi == 0), stop=(mi == 7))
                    for t_ in range(4):
                        acc = y_acc[:, (db * 4 + t_) * 512:(db * 4 + t_ + 1) * 512]
                        if pa == 0:
                            nc.scalar.copy(acc, y_ps[t_][:])
                        else:
                            y_sb = sb_y.tile([128, 512], f32, name="ysb", tag="ysb")
                            nc.vector.tensor_add(y_sb[:], y_ps[t_][:], acc)
                            nc.sync.dma_start(y_d[t_ * 128:(t_ + 1) * 128,
                                                  db * 512:(db + 1) * 512], y_sb[:])

    nc.finalize()
    return nc


def prep_in_maps(x, qkv_w, proj_w):
    """Host-side sharding + bf16 pre-rounding. Returns per-core input maps."""
    import ml_dtypes
    bf = ml_dtypes.bfloat16

    x = np.ascontiguousarray(np.asarray(x, dtype=np.float32).reshape(TOK, C))
    qkv_w = np.asarray(qkv_w, dtype=np.float32)
    proj_w = np.asarray(proj_w, dtype=np.float32)

    xT = np.ascontiguousarray(x.T).astype(bf)     # [C, TOK], shared
    pwT = np.ascontiguousarray(proj_w.T).astype(bf)  # [C, C], shared
    masks = np.zeros((4, 128, 512), dtype=np.float32)
    kj_i = np.arange(128)[:, None]
    qi_i = np.arange(512)[None, :]
    for o in range(4):
        masks[o] = (kj_i + o * 128 <= qi_i).astype(np.float32)
    masks = masks.astype(bf)
    ones2 = np.ones((128, 128), dtype=bf)

    in_maps = []
    for i in range(NCORES):
        r0 = 2 * i * HD
        rows = np.concatenate([
            qkv_w[r0:r0 + 2 * HD],              # q rows, heads 2i, 2i+1
            qkv_w[C + r0:C + r0 + 2 * HD],      # k rows
            qkv_w[2 * C + r0:2 * C + r0 + 2 * HD],  # v rows
        ], axis=0)                              # [768, C]
        wqkvT = np.ascontiguousarray(rows.T).astype(bf)  # [C, 768]
        in_maps.append({"xT": xT, "wqkvT": wqkvT, "pwT": pwT,
                        "masks": masks, "ones2": ones2})
    return in_maps


def kernel(x, qkv_w, proj_w, past=None, past_len=0, **_ignored):
    # past is fully overwritten before being read (past_len == 0), so the
    # output does not depend on it.
    from concourse.bass_utils import run_bass_kernel_spmd
    nc = _CACHE.get("nc")
    if nc is None:
        nc = _CACHE["nc"] = build()
    in_maps = prep_in_maps(x, qkv_w, proj_w)
    res = run_bass_kernel_spmd(nc, in_maps, list(range(NCORES)))
    y = np.concatenate([res.results[i]["y"] for i in range(NCORES)], axis=0)
    return np.ascontiguousarray(y.reshape(B, T, C), dtype=np.float32)
